# revision 29
# baseline (speedup 1.0000x reference)
"""Trainium2 Bass kernel: batched single-head self-attention.

Reference computation (per (b, l) pair, 20 independent blocks):
    X = x[b, l] viewed as [N=1024, D=256] (xf layout)
    out[b, l] = softmax(beta * X @ X.T, axis=-1) @ X

Device algorithm (per block, processed as two independent 512-query
halves so the PSUM accumulators fit):
  * Scores: S[m, n] = sum_d X^T[d, m] X^T[d, n] on the TensorEngine with
    D on partitions (keys m of one 128-row tile on PSUM partitions,
    this half's 512 queries n on the free axis).
  * Softmax shift: W[m, n] = exp(beta * (S[m, n] - c_n)) with
    c_n = ||x_n||^2. The per-QUERY shift rides the score matmul as one
    extra K=128 accumulation term: stationary e1 (partition 0 ones, rest
    zero) broadcasts a host-packed [-c | zeros] chunk of xb to all rows.
    A K=1 ones-row rider works but a row-group-masked matmul forces a
    full PE drain before the next full-array LDWEIGHTS (~430 ns stall).
  * Second matmul, W STATIONARY: for each 128-query block nb of the
    half, out[n, 0:257] = sum_m W[m, n] * [values(m) | 1]. The softmax
    denominator Z_n falls out as output column 256 of the same matmul
    (no separate Z pass), the output leaves in the final [n, d]
    orientation, and each pass streams 257 columns instead of the 512 a
    value-stationary formulation needs. Normalization (divide by Z)
    happens on the host.
  * Dtypes: score operands fp16 (10-bit mantissa; measured end-to-end
    rel err ~4e-3 vs the 2e-2 gate), W tiles / values / outputs bf16
    (W and the unnormalized O reach ~e^60 on contested softmax rows --
    past fp16 range). 16-bit weights let every LDWEIGHTS use the fast
    weight-load path: fp32 weight loads (~224 ns per 128x128, no FWL)
    were the cadence limiter of the fp32r version.
  * Software pipelining: the AV matmuls of key tile a are emitted after
    the score matmuls of tile a+1, so ScalarE's exp of tile a runs under
    the tile-a+1 score streams and the PE never waits on ACT.
  * PSUM: 2 rotating score banks + 6 accumulator banks (4 live per
    query-half + 2 spare so consecutive halves never collide on the
    evacuation).

Sharding: 20 blocks over 8 cores as 2 full blocks + 1 half block (512
queries) per core. The half blocks use a host-side rotation of the key
axis so every core runs the identical program (softmax is invariant to
key permutation when values are permuted identically).
"""

import numpy as np
import ml_dtypes

import concourse.tile as tile
from concourse import bacc, mybir
from concourse.bass_utils import run_bass_kernel_spmd

F32 = mybir.dt.float32
F16 = mybir.dt.float16
BF16 = mybir.dt.bfloat16

B, L, D, H, W = 4, 5, 256, 32, 32
N = H * W            # 1024 keys per block
NBLK = B * L         # 20
NCORES = 8
NFULL = 2            # full blocks per core
NSLAB = 3            # 2 full + 1 half
DF = 272             # value operand row: [x(256) | 1 | 0...] padded so bf16
                     # rows are 544 B = 17x32 B (32 B-aligned weight rows)
DO = 257             # meaningful output row: [O(256) | Z]

EXP = mybir.ActivationFunctionType.Exp


def build_program(beta: float, fast: bool = True):
    sdt = F16 if fast else F32    # score operand dtype
    wdt = BF16 if fast else F32   # W tiles / value operand / output dtype
    nc = bacc.Bacc("TRN2", target_bir_lowering=False, debug=False,
                   num_devices=NCORES)
    # Inputs host-packed in device layout so every DMA is a plain
    # contiguous transfer. xb chunk 2 carries the shift row (-||x_n||^2
    # on partition 0, zeros elsewhere) so it lands with the score
    # operands -- a separate small DMA pays the full ~2.7us DGE launch
    # latency and starves the first riders.
    xb_in = nc.dram_tensor("xb_in", [NSLAB, 3, 128, N], sdt,
                           kind="ExternalInput")
    xf_in = nc.dram_tensor("xf_in", [NSLAB, 128, 8, DF], wdt,
                           kind="ExternalInput")
    # Per (slab, query-half): 128 partition rows x 4 query blocks x
    # [O(256) | Z | pad]; row n = half*512 + nb*128 + p.
    yt_out = nc.dram_tensor("yt_out", [NSLAB, 2, 128, 4, DF], wdt,
                            kind="ExternalOutput")

    with tile.TileContext(nc) as tc:
        _build(tc, nc, xb_in.ap(), xf_in.ap(), yt_out.ap(), beta, sdt, wdt)
    nc.finalize()
    return nc


def _build(tc, nc, xb_in, xf_in, yt_out, beta, sdt, wdt):
    import contextlib
    ctx = contextlib.ExitStack()
    with ctx:
        const = ctx.enter_context(tc.tile_pool(name="const", bufs=1))
        xb_pool = ctx.enter_context(tc.tile_pool(name="xb", bufs=NSLAB))
        xfo_pool = ctx.enter_context(tc.tile_pool(name="xfo", bufs=NSLAB))
        w_pool = ctx.enter_context(tc.tile_pool(name="w", bufs=4))
        ot_pool = ctx.enter_context(tc.tile_pool(name="ot", bufs=3))
        ps_s = ctx.enter_context(tc.tile_pool(name="ps_s", bufs=2,
                                              space="PSUM"))
        ps_acc = ctx.enter_context(tc.tile_pool(name="ps_acc", bufs=6,
                                                space="PSUM"))

        # Warm the PE clock (HAM) with throwaway matmuls that run during
        # the input-DMA window. The first input data only LANDS ~3.5us
        # after the DMA instructions issue (DGE launch latency ~2.7us +
        # transfer), so ~30 short matmuls bridge the whole window; by the
        # time real work starts the clock is at full rate. warm_src is
        # deliberately NOT initialized -- garbage (even NaN) products land
        # in a discarded PSUM bank whose next user writes with start=True,
        # and skipping the memset lets the warmups issue ~1us earlier.
        warm_src = const.tile([128, 128], sdt)
        # single-partition memset: allocates the tile (the framework
        # rejects read-only tiles) at ~1/128th the cost of a full zero
        nc.gpsimd.memset(warm_src[0:1, :], 0.0)
        warm_ps = ps_acc.tile([128, 512], F32, tag="acc", name="warm_ps")
        for wi in range(30):
            nc.tensor.matmul(warm_ps[:, 0:128], warm_src[:], warm_src[:],
                             start=True, stop=True)

        # Shift-rider stationary (see module docstring).
        e1 = const.tile([128, 128], sdt)
        nc.gpsimd.memset(e1[:], 0.0)
        nc.gpsimd.memset(e1[0:1, :], 1.0)

        # All input DMAs upfront, spread over the two hardware DGE
        # queues so slab 0 lands first.
        xbs, xfos = [], []
        for s in range(NSLAB):
            xb = xb_pool.tile([128, 3, N], sdt, tag="xb", name=f"xb_{s}")
            xbs.append(xb)
        nc.sync.dma_start(out=xbs[0][:, 0, :], in_=xb_in[0][0])
        nc.scalar.dma_start(out=xbs[0][:, 1, :], in_=xb_in[0][1])
        # shift-row chunk split per query-half: the first rider only
        # needs columns 0:512, which then land ~0.35us after chunk 0
        nc.sync.dma_start(out=xbs[0][:, 2, 0:512], in_=xb_in[0][2][:, 0:512])
        nc.scalar.dma_start(out=xbs[0][:, 2, 512:N],
                            in_=xb_in[0][2][:, 512:N])
        for s in (1, 2):
            nc.sync.dma_start(
                out=xbs[s][:], in_=xb_in[s].rearrange("c p n -> p c n"))
        for s in range(NSLAB):
            xfo = xfo_pool.tile([128, 8, DF], wdt, tag="xfo",
                                name=f"xfo_{s}")
            nc.scalar.dma_start(out=xfo[:], in_=xf_in[s])
            xfos.append(xfo)

        # Emission order, 1-deep pipelined ACROSS group boundaries (a
        # group = one slab query-half). The last AV of group g is
        # deferred past the first scores of group g+1, so the exp at a
        # group start always has score streams to hide under -- a
        # per-group pipeline drains at every boundary and costs ~1.5 us
        # in exp-wait plus a HAM re-throttle:
        #   ... scores(g,7) av(g,6) | scores(g+1,0) av(g,7) evac(g)
        #   scores(g+1,1) av(g+1,0) ...
        groups = [(s, half) for s in range(NSLAB)
                  for half in range(2 if s < NFULL else 1)]
        flush_prev = None
        for group, (s, half) in enumerate(groups):
            xb, xfo = xbs[s], xfos[s]
            hs = slice(half * 512, (half + 1) * 512)
            acc = None
            w_tiles = []

            def emit_av(a, acc=None, w_tiles=None, xfo=xfo):
                for nb in range(4):
                    nc.tensor.matmul(acc[nb][:, 0:DO],
                                     w_tiles[a][:, nb * 128:(nb + 1) * 128],
                                     xfo[:, a, 0:DO],
                                     start=(a == 0), stop=(a == 7))

            def finish(acc, group=group, s=s, half=half,
                       last=(group == len(groups) - 1)):
                # Evacuate [O | Z] to SBUF, then DMA out. Mid-kernel
                # groups keep 3 copies on DVE (ACT's exp queue must stay
                # clear -- the next group's AV matmuls wait on it); the
                # last group has no more exps, so it balances 2/2 and
                # departs in four per-block transfers so the final DGE
                # launch starts as early as possible.
                ot = ot_pool.tile([128, 4, DF], wdt, tag="ot",
                                  name=f"ot_{group}")
                oth = ot_pool.tile([128, 4, DF], wdt, tag="oth",
                                   name=f"oth_{group}")
                n_dve = 2 if last else 3
                for nb in range(4):
                    dst = (ot if nb < 2 else oth)[:, nb, 0:DO]
                    if nb < n_dve:
                        nc.vector.tensor_copy(dst, acc[nb][:, 0:DO])
                    else:
                        nc.scalar.copy(dst, acc[nb][:, 0:DO])
                eng = nc.sync if group % 2 == 0 else nc.scalar
                eng2 = nc.scalar if group % 2 == 0 else nc.sync
                if last:
                    eng.dma_start(out=yt_out[s][half][:, 0:1],
                                  in_=ot[:, 0:1])
                    eng2.dma_start(out=yt_out[s][half][:, 2:3],
                                   in_=oth[:, 2:3])
                    eng.dma_start(out=yt_out[s][half][:, 1:2],
                                  in_=ot[:, 1:2])
                    eng2.dma_start(out=yt_out[s][half][:, 3:4],
                                   in_=oth[:, 3:4])
                else:
                    eng.dma_start(out=yt_out[s][half][:, 0:2],
                                  in_=ot[:, 0:2])
                    eng2.dma_start(out=yt_out[s][half][:, 2:4],
                                   in_=oth[:, 2:4])

            for a in range(8):
                asl = slice(a * 128, (a + 1) * 128)
                sps = ps_s.tile([128, 512], F32, tag="sps",
                                name=f"sps_{group}_{a}")
                for c in range(2):
                    nc.tensor.matmul(sps[:], xb[:, c, asl], xb[:, c, hs],
                                     start=(c == 0), stop=False)
                nc.tensor.matmul(sps[:], e1[:], xb[:, 2, hs],
                                 start=False, stop=True)
                wt = w_pool.tile([128, 512], wdt, tag="w",
                                 name=f"w_{group}_{a}")
                nc.scalar.activation(wt[:], sps[:], EXP, scale=float(beta))
                w_tiles.append(wt)
                if a == 0:
                    if flush_prev is not None:
                        flush_prev()
                        flush_prev = None
                    elif group == 0:
                        # bridge the first group's pipeline-fill bubble
                        # (exp latency with nothing to overlap) so the
                        # HAM clock monitor sees continuous PE activity
                        for wi in range(6):
                            nc.tensor.matmul(warm_ps[:, 0:128],
                                             warm_src[:], warm_src[:],
                                             start=True, stop=True)
                else:
                    if acc is None:
                        acc = [ps_acc.tile([128, 512], F32, tag="acc",
                                           name=f"acc_{group}_{nb}")
                               for nb in range(4)]
                    emit_av(a - 1, acc, w_tiles)
            def flush_prev(acc=acc, wts=w_tiles, fin=finish, av=emit_av):
                av(7, acc, wts)
                fin(acc)
        flush_prev()


_PROG_CACHE = {}


def _get_program(beta: float, fast: bool = True):
    key = (beta, fast)
    if key not in _PROG_CACHE:
        _PROG_CACHE[key] = build_program(beta, fast)
    return _PROG_CACHE[key]


def make_in_maps(x: np.ndarray, fast: bool = True):
    """Shard the full input [B, L, D, H, W] into 8 per-core input maps."""
    sdt = np.float16 if fast else np.float32
    wdt = ml_dtypes.bfloat16 if fast else np.float32
    xt_all = np.ascontiguousarray(x.reshape(NBLK, D, N))
    in_maps = []
    for c in range(NCORES):
        half_blk = NFULL * NCORES + c // 2
        half = xt_all[half_blk]
        if c % 2 == 1:
            # rotate keys so this core's queries are columns 0..511
            half = np.concatenate([half[:, N // 2:], half[:, :N // 2]], axis=1)
        slabs = np.stack([xt_all[NFULL * c], xt_all[NFULL * c + 1], half])
        slabs16 = slabs.astype(sdt)
        # shift row from the rounded operands (any per-query shift cancels
        # exactly in O/Z; using the rounded data keeps the overflow margin)
        s32 = slabs16.astype(np.float32)
        negc = -np.einsum('sdn,sdn->sn', s32, s32)
        xf = np.zeros((NSLAB, N, DF), np.float32)
        xf[:, :, :D] = slabs.transpose(0, 2, 1)
        xf[:, :, D] = 1.0
        # pack into device layout: xb [3, 128, N] (chunk 2 = shift row on
        # partition 0, zeros below), xf [128, 8, DF]
        xb_p = np.zeros((NSLAB, 3, 128, N), sdt)
        xb_p[:, 0:2] = slabs16.reshape(NSLAB, 2, 128, N)
        xb_p[:, 2, 0, :] = negc.astype(sdt)
        xf_p = np.ascontiguousarray(
            xf.reshape(NSLAB, 8, 128, DF).transpose(0, 2, 1, 3)).astype(wdt)
        in_maps.append({"xb_in": np.ascontiguousarray(xb_p),
                        "xf_in": xf_p})
    return in_maps


def assemble_output(results):
    """Normalize and gather per-core outputs into [B, L, N, D]."""
    out = np.empty((NBLK, N, D), np.float32)
    for c in range(NCORES):
        yt = np.asarray(results[c]["yt_out"], dtype=np.float32)
        for s, blk, lo, n_q in ((0, NFULL * c, 0, N),
                                (1, NFULL * c + 1, 0, N),
                                (2, NFULL * NCORES + c // 2,
                                 (c % 2) * (N // 2), N // 2)):
            for h in range(n_q // 512):
                seg = yt[s, h]                        # [128, 4, DF]
                o = seg[:, :, 0:D]
                z = seg[:, :, D]
                rows = (o / z[..., None]).transpose(1, 0, 2).reshape(512, D)
                base = lo + h * 512
                out[blk, base:base + 512] = rows
    return out.reshape(B, L, N, D)


def kernel(x, beta, _trace=False, _fast=True, _tmpdir=None):
    x = np.asarray(x, dtype=np.float32)
    assert x.shape == (B, L, D, H, W), x.shape
    beta_f = float(np.asarray(beta))
    prog = _get_program(beta_f, _fast)
    in_maps = make_in_maps(x, _fast)
    res = run_bass_kernel_spmd(prog, in_maps, core_ids=list(range(NCORES)),
                               trace=_trace, tmpdir=_tmpdir)
    out = assemble_output(res.results)
    if _trace:
        return out, res
    return out


# revision 30
# speedup vs baseline: 1.0607x; 1.0607x over previous
"""Trainium2 Bass kernel: batched single-head self-attention.

Reference computation (per (b, l) pair, 20 independent blocks):
    X = x[b, l] viewed as [N=1024, D=256] (xf layout)
    out[b, l] = softmax(beta * X @ X.T, axis=-1) @ X

Device algorithm (per block, processed as two independent 512-query
halves so the PSUM accumulators fit):
  * Scores: S[m, n] = sum_d X^T[d, m] X^T[d, n] on the TensorEngine with
    D on partitions (keys m of one 128-row tile on PSUM partitions,
    this half's 512 queries n on the free axis).
  * Softmax shift: W[m, n] = exp(beta * (S[m, n] - c_n)) with
    c_n = ||x_n||^2. The per-QUERY shift rides the score matmul as one
    extra K=128 accumulation term: stationary e1 (partition 0 ones, rest
    zero) broadcasts a host-packed [-c | zeros] chunk of xb to all rows.
    A K=1 ones-row rider works but a row-group-masked matmul forces a
    full PE drain before the next full-array LDWEIGHTS (~430 ns stall).
  * Second matmul, W STATIONARY: for each 128-query block nb of the
    half, out[n, 0:257] = sum_m W[m, n] * [values(m) | 1]. The softmax
    denominator Z_n falls out as output column 256 of the same matmul
    (no separate Z pass), the output leaves in the final [n, d]
    orientation, and each pass streams 257 columns instead of the 512 a
    value-stationary formulation needs. Normalization (divide by Z)
    happens on the host.
  * Dtypes: score operands fp16 (10-bit mantissa; measured end-to-end
    rel err ~4e-3 vs the 2e-2 gate), W tiles / values / outputs bf16
    (W and the unnormalized O reach ~e^60 on contested softmax rows --
    past fp16 range). 16-bit weights let every LDWEIGHTS use the fast
    weight-load path: fp32 weight loads (~224 ns per 128x128, no FWL)
    were the cadence limiter of the fp32r version.
  * Software pipelining: the AV matmuls of key tile a are emitted after
    the score matmuls of tile a+1, so ScalarE's exp of tile a runs under
    the tile-a+1 score streams and the PE never waits on ACT.
  * PSUM: 2 rotating score banks + 6 accumulator banks (4 live per
    query-half + 2 spare so consecutive halves never collide on the
    evacuation).

Sharding: 20 blocks over 8 cores as 2 full blocks + 1 half block (512
queries) per core. The half blocks use a host-side rotation of the key
axis so every core runs the identical program (softmax is invariant to
key permutation when values are permuted identically).
"""

import numpy as np
import ml_dtypes

import concourse.tile as tile
from concourse import bacc, mybir
from concourse.bass_utils import run_bass_kernel_spmd

F32 = mybir.dt.float32
F16 = mybir.dt.float16
BF16 = mybir.dt.bfloat16

B, L, D, H, W = 4, 5, 256, 32, 32
N = H * W            # 1024 keys per block
NBLK = B * L         # 20
NCORES = 8
NFULL = 2            # full blocks per core
NSLAB = 3            # 2 full + 1 half
DF = 272             # value operand row: [x(256) | 1 | 0...] padded so bf16
                     # rows are 544 B = 17x32 B (32 B-aligned weight rows)
DO = 257             # meaningful output row: [O(256) | Z]

EXP = mybir.ActivationFunctionType.Exp


def build_program(beta: float, fast: bool = True):
    sdt = F16 if fast else F32    # score operand dtype
    wdt = BF16 if fast else F32   # W tiles / value operand / output dtype
    nc = bacc.Bacc("TRN2", target_bir_lowering=False, debug=False,
                   num_devices=NCORES)
    # Inputs host-packed in device layout so every DMA is a plain
    # contiguous transfer. xb chunk 2 carries the shift row (-||x_n||^2
    # on partition 0, zeros elsewhere) so it lands with the score
    # operands -- a separate small DMA pays the full ~2.7us DGE launch
    # latency and starves the first riders.
    xb_in = nc.dram_tensor("xb_in", [NSLAB, 3, 128, N], sdt,
                           kind="ExternalInput")
    xf_in = nc.dram_tensor("xf_in", [NSLAB, 128, 8, DF], wdt,
                           kind="ExternalInput")
    # Per (slab, query-half): 128 partition rows x 4 query blocks x
    # [O(256) | Z | pad]; row n = half*512 + nb*128 + p.
    yt_out = nc.dram_tensor("yt_out", [NSLAB, 2, 128, 4, DF], wdt,
                            kind="ExternalOutput")

    with tile.TileContext(nc) as tc:
        _build(tc, nc, xb_in.ap(), xf_in.ap(), yt_out.ap(), beta, sdt, wdt)
    nc.finalize()
    return nc


def _build(tc, nc, xb_in, xf_in, yt_out, beta, sdt, wdt):
    import contextlib
    ctx = contextlib.ExitStack()
    with ctx:
        const = ctx.enter_context(tc.tile_pool(name="const", bufs=1))
        xb_pool = ctx.enter_context(tc.tile_pool(name="xb", bufs=NSLAB))
        xfo_pool = ctx.enter_context(tc.tile_pool(name="xfo", bufs=NSLAB))
        w_pool = ctx.enter_context(tc.tile_pool(name="w", bufs=4))
        ot_pool = ctx.enter_context(tc.tile_pool(name="ot", bufs=3))
        ps_s = ctx.enter_context(tc.tile_pool(name="ps_s", bufs=2,
                                              space="PSUM"))
        ps_acc = ctx.enter_context(tc.tile_pool(name="ps_acc", bufs=6,
                                                space="PSUM"))

        # Warm the PE clock (HAM) with throwaway matmuls that run during
        # the input-DMA window. The first input data only LANDS ~3.5us
        # after the DMA instructions issue (DGE launch latency ~2.7us +
        # transfer), so ~30 short matmuls bridge the whole window; by the
        # time real work starts the clock is at full rate. warm_src is
        # deliberately NOT initialized -- garbage (even NaN) products land
        # in a discarded PSUM bank whose next user writes with start=True,
        # and skipping the memset lets the warmups issue ~1us earlier.
        warm_src = const.tile([128, 128], sdt)
        # single-partition memset: allocates the tile (the framework
        # rejects read-only tiles) at ~1/128th the cost of a full zero
        nc.gpsimd.memset(warm_src[0:1, :], 0.0)
        warm_ps = ps_acc.tile([128, 512], F32, tag="acc", name="warm_ps")
        for wi in range(30):
            nc.tensor.matmul(warm_ps[:, 0:128], warm_src[:], warm_src[:],
                             start=True, stop=True)

        # Shift-rider stationary (see module docstring).
        e1 = const.tile([128, 128], sdt)
        nc.gpsimd.memset(e1[:], 0.0)
        nc.gpsimd.memset(e1[0:1, :], 1.0)

        # All input DMAs upfront, spread over the two hardware DGE
        # queues so slab 0 lands first.
        xbs, xfos = [], []
        for s in range(NSLAB):
            xb = xb_pool.tile([128, 3, N], sdt, tag="xb", name=f"xb_{s}")
            xbs.append(xb)
        nc.sync.dma_start(out=xbs[0][:, 0, :], in_=xb_in[0][0])
        nc.scalar.dma_start(out=xbs[0][:, 1, :], in_=xb_in[0][1])
        nc.sync.dma_start(out=xbs[0][:, 2, :], in_=xb_in[0][2])
        for s in (1, 2):
            nc.sync.dma_start(
                out=xbs[s][:], in_=xb_in[s].rearrange("c p n -> p c n"))
        for s in range(NSLAB):
            xfo = xfo_pool.tile([128, 8, DF], wdt, tag="xfo",
                                name=f"xfo_{s}")
            nc.scalar.dma_start(out=xfo[:], in_=xf_in[s])
            xfos.append(xfo)

        # Emission order, 1-deep pipelined ACROSS group boundaries (a
        # group = one slab query-half). The last AV of group g is
        # deferred past the first scores of group g+1, so the exp at a
        # group start always has score streams to hide under -- a
        # per-group pipeline drains at every boundary and costs ~1.5 us
        # in exp-wait plus a HAM re-throttle:
        #   ... scores(g,7) av(g,6) | scores(g+1,0) av(g,7) evac(g)
        #   scores(g+1,1) av(g+1,0) ...
        groups = [(s, half) for s in range(NSLAB)
                  for half in range(2 if s < NFULL else 1)]
        flush_prev = None
        for group, (s, half) in enumerate(groups):
            xb, xfo = xbs[s], xfos[s]
            hs = slice(half * 512, (half + 1) * 512)
            acc = None
            w_tiles = []

            def emit_av(a, acc=None, w_tiles=None, xfo=xfo):
                for nb in range(4):
                    nc.tensor.matmul(acc[nb][:, 0:DO],
                                     w_tiles[a][:, nb * 128:(nb + 1) * 128],
                                     xfo[:, a, 0:DO],
                                     start=(a == 0), stop=(a == 7))

            def finish(acc, group=group, s=s, half=half,
                       last=(group == len(groups) - 1)):
                # Evacuate [O | Z] to SBUF, then DMA out. Mid-kernel
                # groups keep 3 copies on DVE (ACT's exp queue must stay
                # clear -- the next group's AV matmuls wait on it); the
                # last group has no more exps, so it balances 2/2 and
                # departs in four per-block transfers so the final DGE
                # launch starts as early as possible.
                ot = ot_pool.tile([128, 4, DF], wdt, tag="ot",
                                  name=f"ot_{group}")
                oth = ot_pool.tile([128, 4, DF], wdt, tag="oth",
                                   name=f"oth_{group}")
                n_dve = 2 if last else 3
                for nb in range(4):
                    dst = (ot if nb < 2 else oth)[:, nb, 0:DO]
                    if nb < n_dve:
                        nc.vector.tensor_copy(dst, acc[nb][:, 0:DO])
                    else:
                        nc.scalar.copy(dst, acc[nb][:, 0:DO])
                eng = nc.sync if group % 2 == 0 else nc.scalar
                eng2 = nc.scalar if group % 2 == 0 else nc.sync
                if last:
                    eng.dma_start(out=yt_out[s][half][:, 0:1],
                                  in_=ot[:, 0:1])
                    eng2.dma_start(out=yt_out[s][half][:, 2:3],
                                   in_=oth[:, 2:3])
                    eng.dma_start(out=yt_out[s][half][:, 1:2],
                                  in_=ot[:, 1:2])
                    eng2.dma_start(out=yt_out[s][half][:, 3:4],
                                   in_=oth[:, 3:4])
                else:
                    eng.dma_start(out=yt_out[s][half][:, 0:2],
                                  in_=ot[:, 0:2])
                    eng2.dma_start(out=yt_out[s][half][:, 2:4],
                                   in_=oth[:, 2:4])

            for a in range(8):
                asl = slice(a * 128, (a + 1) * 128)
                sps = ps_s.tile([128, 512], F32, tag="sps",
                                name=f"sps_{group}_{a}")
                for c in range(2):
                    nc.tensor.matmul(sps[:], xb[:, c, asl], xb[:, c, hs],
                                     start=(c == 0), stop=False)
                nc.tensor.matmul(sps[:], e1[:], xb[:, 2, hs],
                                 start=False, stop=True)
                wt = w_pool.tile([128, 512], wdt, tag="w",
                                 name=f"w_{group}_{a}")
                nc.scalar.activation(wt[:], sps[:], EXP, scale=float(beta))
                w_tiles.append(wt)
                if a == 0:
                    if flush_prev is not None:
                        flush_prev()
                        flush_prev = None
                    elif group == 0:
                        # bridge the first group's pipeline-fill bubble
                        # (exp latency with nothing to overlap) so the
                        # HAM clock monitor sees continuous PE activity
                        for wi in range(6):
                            nc.tensor.matmul(warm_ps[:, 0:128],
                                             warm_src[:], warm_src[:],
                                             start=True, stop=True)
                else:
                    if acc is None:
                        acc = [ps_acc.tile([128, 512], F32, tag="acc",
                                           name=f"acc_{group}_{nb}")
                               for nb in range(4)]
                    emit_av(a - 1, acc, w_tiles)
            def flush_prev(acc=acc, wts=w_tiles, fin=finish, av=emit_av):
                av(7, acc, wts)
                fin(acc)
        flush_prev()


_PROG_CACHE = {}


def _get_program(beta: float, fast: bool = True):
    key = (beta, fast)
    if key not in _PROG_CACHE:
        _PROG_CACHE[key] = build_program(beta, fast)
    return _PROG_CACHE[key]


def make_in_maps(x: np.ndarray, fast: bool = True):
    """Shard the full input [B, L, D, H, W] into 8 per-core input maps."""
    sdt = np.float16 if fast else np.float32
    wdt = ml_dtypes.bfloat16 if fast else np.float32
    xt_all = np.ascontiguousarray(x.reshape(NBLK, D, N))
    in_maps = []
    for c in range(NCORES):
        half_blk = NFULL * NCORES + c // 2
        half = xt_all[half_blk]
        if c % 2 == 1:
            # rotate keys so this core's queries are columns 0..511
            half = np.concatenate([half[:, N // 2:], half[:, :N // 2]], axis=1)
        slabs = np.stack([xt_all[NFULL * c], xt_all[NFULL * c + 1], half])
        slabs16 = slabs.astype(sdt)
        # shift row from the rounded operands (any per-query shift cancels
        # exactly in O/Z; using the rounded data keeps the overflow margin)
        s32 = slabs16.astype(np.float32)
        negc = -np.einsum('sdn,sdn->sn', s32, s32)
        xf = np.zeros((NSLAB, N, DF), np.float32)
        xf[:, :, :D] = slabs.transpose(0, 2, 1)
        xf[:, :, D] = 1.0
        # pack into device layout: xb [3, 128, N] (chunk 2 = shift row on
        # partition 0, zeros below), xf [128, 8, DF]
        xb_p = np.zeros((NSLAB, 3, 128, N), sdt)
        xb_p[:, 0:2] = slabs16.reshape(NSLAB, 2, 128, N)
        xb_p[:, 2, 0, :] = negc.astype(sdt)
        xf_p = np.ascontiguousarray(
            xf.reshape(NSLAB, 8, 128, DF).transpose(0, 2, 1, 3)).astype(wdt)
        in_maps.append({"xb_in": np.ascontiguousarray(xb_p),
                        "xf_in": xf_p})
    return in_maps


def assemble_output(results):
    """Normalize and gather per-core outputs into [B, L, N, D]."""
    out = np.empty((NBLK, N, D), np.float32)
    for c in range(NCORES):
        yt = np.asarray(results[c]["yt_out"], dtype=np.float32)
        for s, blk, lo, n_q in ((0, NFULL * c, 0, N),
                                (1, NFULL * c + 1, 0, N),
                                (2, NFULL * NCORES + c // 2,
                                 (c % 2) * (N // 2), N // 2)):
            for h in range(n_q // 512):
                seg = yt[s, h]                        # [128, 4, DF]
                o = seg[:, :, 0:D]
                z = seg[:, :, D]
                rows = (o / z[..., None]).transpose(1, 0, 2).reshape(512, D)
                base = lo + h * 512
                out[blk, base:base + 512] = rows
    return out.reshape(B, L, N, D)


def kernel(x, beta, _trace=False, _fast=True, _tmpdir=None):
    x = np.asarray(x, dtype=np.float32)
    assert x.shape == (B, L, D, H, W), x.shape
    beta_f = float(np.asarray(beta))
    prog = _get_program(beta_f, _fast)
    in_maps = make_in_maps(x, _fast)
    res = run_bass_kernel_spmd(prog, in_maps, core_ids=list(range(NCORES)),
                               trace=_trace, tmpdir=_tmpdir)
    out = assemble_output(res.results)
    if _trace:
        return out, res
    return out
